# revision 67
# baseline (speedup 1.0000x reference)
"""Trainium2 Bass kernel for nn_AttentionBlock (B=4, N=2048, IN_C=256, H=8,
HEAD_C=32, EXPAND=1024), distributed over 8 NeuronCores.

Sharding: data-parallel over query rows. Core c handles batch c//2, query rows
(c%2)*1024 .. +1024 of that batch. Each core receives its batch's tokens in
PERMUTED order (its own query rows first) so the query half of the LayerNorm /
transpose work doubles as the key/value half — softmax over keys is
permutation-invariant, so kT/ve just use the permuted order consistently.

Precision/engine strategy:
 - LN gains/betas are folded into the projection weights/biases host-side, so
   on-chip LN is just (x - mean) * rsqrt(var), evicted straight to fp8e4.
   rsqrt = exp(-0.5*ln(v+eps)) keeps ScalarE in ONE activation-table set
   (natural_log_exp, forced via the get_activation_tables patch below) for
   the whole LN+softmax program — table reloads cost ~1.5us each.
 - Q/K/V projections run as fp8 DoubleRow matmuls (2 k-tiles per pass).
 - Scores are bf16 in (128,128) tile mode: qT is per-head ZERO-PADDED to 128
   rows (row band 64*(h%2)..+32) so the score matmul contracts over the full
   128 rows of the packed kT[h//2] block — dead rows hit zeros. This keeps
   every PE matmul in the same tiling mode (no drain between scores and the
   fp8-DR PV/ve matmuls, which cost ~300ns each in the (32,128) variant).
 - Softmax: exp(S - 4) from PSUM to fp8e4 (S max ~8.6 so exp stays below the
   e4m3 240 cap; the -4 offset cancels in normalization). The eviction is
   load-balanced per key tile: ScalarE exp for most tiles, a Schraudolph
   bitcast-exp (i32 affine on DVE, downcast on GpSimd or DVE) for the rest —
   GPSIMD cannot read PSUM, so DVE always runs the PSUM-side pass.
 - PV runs as fp8 DoubleRow over key-tile pairs; the appended "ones" column
   is set to 16.0, cancelling the x16 scaling folded into wvm (which keeps
   wvm out of the fp8 subnormal range) at normalization time.
 - FFN stays bf16 end to end for accuracy.
 - Heads are software-pipelined with scores leading PV by two j-slots (exps
   rotate through 3 buffers): every eviction has ~3 slots of queue slack
   before PV(h, 0) consumes the full head, hiding ScalarE/GpSimd latency.
 - Tail streams the second query half's PV/LN2 between FFN1 chunks so gelu
   (ScalarE) and the PE stay co-busy.
"""
import numpy as np
import ml_dtypes

import concourse.bass as bass
import concourse.bacc as bacc
import concourse.tile as tile
from concourse import mybir
from concourse.bass_utils import run_bass_kernel_spmd
from concourse.masks import make_identity

F32 = mybir.dt.float32
BF16 = mybir.dt.bfloat16
FP8 = mybir.dt.float8e4
I32 = mybir.dt.int32
AF = mybir.ActivationFunctionType
ALU = mybir.AluOpType
DR = mybir.MatmulPerfMode.DoubleRow
P = 128

# The act-table insertion pass picks the FIRST table set containing each
# activation function. Ln lives in `natural_log` (set 5) and Exp in
# `exp_and_others` (set 0), so a kernel alternating Ln/Exp reloads tables
# (~1.5us each) on every switch. Restrict membership so both resolve to the
# combined `natural_log_exp_and_others` set: the emitted act_func_set_id
# still indexes the real act_info.json list, so runtime tables are correct.
_GAT_PATCHED = False


def _patch_act_tables():
    global _GAT_PATCHED
    if _GAT_PATCHED:
        return
    orig = bacc.get_activation_tables

    def patched(arch):
        t = dict(orig(arch))
        for name in ("exp_and_others", "exp_and_friends"):
            if name in t:
                t[name] = set(t[name]) - {AF.Exp}
        if "natural_log" in t:
            t["natural_log"] = set(t["natural_log"]) - {AF.Ln}
        return t

    bacc.get_activation_tables = patched
    _GAT_PATCHED = True

B, N, IN_C, HEAD_C, H, EXPAND = 4, 2048, 256, 32, 8, 1024
NQ = 1024            # query rows per core

# softmax-exp engine assignment per key tile: Schraudolph offload relieves
# ScalarE, which is otherwise the attention-era bottleneck. 'pipe' = DVE
# affine pass + GpSimd downcast; 'dve' = both passes on DVE.
# kc 0-3 (prefetched across the head boundary) and kc 15 stay on ACT: a
# Schraudolph pass1 queued on DVE at the boundary delays the ve/normalize
# items the PE is about to need (DVE is a strict FIFO), stalling PV(h, 0).
_PIPE_KC = frozenset({5, 8, 11})
_DVE_KC = frozenset({6, 14})


def ENG_FOR(h, kc):
    if kc in _PIPE_KC:
        return "pipe"
    if kc in _DVE_KC:
        return "dve"
    return "act"
NT_B = N // P        # 16 token tiles per batch
NT_Q = NQ // P       # 8 query tiles per core
LN_EPS = 1e-5
C_EXP = 4.0          # exp offset: exp(S - C_EXP) <= ~110 < 240 (e4m3 max)
VE_W = IN_C + 1      # 257: value channels + sum column (holds 16.0)
SCALE = HEAD_C ** -0.5

_NC = None  # cached compiled graph


def _build(do_compile=True):
    _patch_act_tables()
    nc = bacc.Bacc("TRN2", target_bir_lowering=False, debug=False, num_devices=8)

    xb_e = nc.dram_tensor("xb", [N, IN_C], F32, kind="ExternalInput")
    # fp8 weights, contraction-pair layout: free dim = (k, cols), k in {0,1}
    # selecting input-channel chunk k*128..+128.
    wq_e = nc.dram_tensor("wq", [P, 2 * H * P], FP8, kind="ExternalInput")
    wk_e = nc.dram_tensor("wk", [P, 2 * 4 * P], FP8, kind="ExternalInput")
    wvm_e = nc.dram_tensor("wvm", [P, 2 * H * IN_C], FP8, kind="ExternalInput")
    w1_e = nc.dram_tensor("w1", [P, 2 * EXPAND], BF16, kind="ExternalInput")
    w2_e = nc.dram_tensor("w2", [P, EXPAND // P * IN_C], BF16, kind="ExternalInput")
    battn_e = nc.dram_tensor("battn", [IN_C], F32, kind="ExternalInput")
    b2_e = nc.dram_tensor("b2", [IN_C], F32, kind="ExternalInput")
    bq_e = nc.dram_tensor("bq", [P, H], F32, kind="ExternalInput")
    b1_e = nc.dram_tensor("b1", [P, EXPAND // P], F32, kind="ExternalInput")
    out_es = [nc.dram_tensor(f"out{i}", [P, IN_C], F32, kind="ExternalOutput")
              for i in range(NT_Q)]

    from contextlib import ExitStack
    with tile.TileContext(nc) as tc, ExitStack() as ctx:
        const = ctx.enter_context(tc.tile_pool(name="const", bufs=1))
        persist = ctx.enter_context(tc.tile_pool(name="persist", bufs=1))
        lnt = ctx.enter_context(tc.tile_pool(name="lnt", bufs=3))
        stats_p = ctx.enter_context(tc.tile_pool(name="stats", bufs=4))
        on_p = ctx.enter_context(tc.tile_pool(name="on", bufs=3))
        st_ps = ctx.enter_context(tc.tile_pool(name="st_ps", bufs=2, space="PSUM"))
        o_ps = ctx.enter_context(tc.tile_pool(name="o_ps", bufs=2, space="PSUM"))
        mm_ps = ctx.enter_context(tc.tile_pool(name="mm_ps", bufs=2, space="PSUM"))

        # ---- input + early-needed weights first: the LN1 chain starts on
        # xin immediately and kproj/qproj need wk/wq within ~3us; wvm/w1/w2
        # are needed later and must not crowd the queues. ----
        wq_t = const.tile([P, 2 * H * P], FP8, tag="wq", name="wq")
        nc.sync.dma_start(out=wq_t[:], in_=wq_e[:])
        wk_t = const.tile([P, 2 * 4 * P], FP8, tag="wk", name="wk")
        nc.sync.dma_start(out=wk_t[:], in_=wk_e[:])
        xin = persist.tile([P, NT_B * IN_C], F32, tag="xin", name="xin")
        for t in range(NT_B):
            nc.sync.dma_start(
                out=xin[:, t * IN_C:(t + 1) * IN_C],
                in_=xb_e[t * P:(t + 1) * P, :])

        # ---- constants ----
        def rep256(name, ext):
            t = const.tile([P, IN_C], F32, tag=name, name=name)
            nc.sync.dma_start(
                out=t[:], in_=ext.ap()[None, :].broadcast_to([P, IN_C]))
            return t

        battn_r = rep256("battn", battn_e)
        b2_r = rep256("b2", b2_e)
        bq_t = const.tile([P, H], F32, tag="bq", name="bq")
        nc.sync.dma_start(out=bq_t[:], in_=bq_e[:])
        b1_t = const.tile([P, EXPAND // P], F32, tag="b1", name="b1")
        nc.sync.dma_start(out=b1_t[:], in_=b1_e[:])
        eps_t = const.tile([P, 1], F32, tag="eps", name="eps")
        nc.vector.memset(eps_t[:], LN_EPS)
        cexp_t = const.tile([P, 1], F32, tag="cexp", name="cexp")
        nc.vector.memset(cexp_t[:], -C_EXP)
        id16 = const.tile([P, P], BF16, tag="id16", name="id16")
        make_identity(nc, id16[:])

        wvm_t = const.tile([P, 2 * H * IN_C], FP8, tag="wvm", name="wvm")
        nc.sync.dma_start(out=wvm_t[:], in_=wvm_e[:])
        w1_t = const.tile([P, 2 * EXPAND], BF16, tag="w1", name="w1")
        nc.sync.dma_start(out=w1_t[:], in_=w1_e[:])
        w2_t = const.tile([P, EXPAND // P * IN_C], BF16, tag="w2", name="w2")
        nc.sync.dma_start(out=w2_t[:], in_=w2_e[:])
        wq3 = wq_t[:].rearrange("p (k n) -> p k n", k=2)
        wk3 = wk_t[:].rearrange("p (k n) -> p k n", k=2)
        wvm3 = wvm_t[:].rearrange("p (k n) -> p k n", k=2)

        # ---- persistent activations ----
        lnT = persist.tile([P, 2 * N], FP8, tag="lnT", name="lnT")
        lnT3 = lnT[:].rearrange("p (k n) -> p k n", k=2)
        # per-head zero-padded qT: head h occupies rows 64*(h%2)..+32, rest 0.
        # Scores then contract over the FULL 128 rows of packed kT[h//2]
        # (whose other rows hold the partner head / zeros), keeping every
        # matmul in (128,128) tile mode - no PE tiling-mode drains.
        qT = [persist.tile([P, NQ], BF16, tag=f"qT{h}", name=f"qT{h}")
              for h in range(H)]
        kT = [persist.tile([P, N], BF16, tag=f"kT{k}", name=f"kT{k}")
              for k in range(4)]
        x2acc = [persist.tile([P, IN_C], F32, tag=f"x2acc{i}", name=f"x2acc{i}")
                 for i in range(NT_Q)]
        x2T = persist.tile([P, 2 * NQ], BF16, tag="x2T", name="x2T")
        hT = persist.tile([P, EXPAND // P * NQ], BF16, tag="hT", name="hT")
        exps = [persist.tile([P, NT_B * NQ], FP8, tag=f"exps{i}", name=f"exps{i}")
                for i in range(3)]
        ve = [persist.tile([P, NT_B * VE_W], FP8, tag=f"ve{i}", name=f"ve{i}")
              for i in range(3)]
        # sum columns hold 16.0: cancels the x16 folded into wvm at normalize
        for i in range(3):
            nc.gpsimd.memset(
                ve[i][:].rearrange("p (t c) -> p t c", c=VE_W)[:, :, IN_C:IN_C + 1],
                16.0)
        ex3 = [e[:].rearrange("p (k n) -> p k n", k=NT_B) for e in exps]
        ve3 = [v[:].rearrange("p (t c) -> p t c", c=VE_W) for v in ve]



        # ---- LayerNorm: stats on DVE, sqrt on ACT, normalize straight to out
        def ln_norm(x_ap, out_dtype, tag):
            # rsqrt(v+eps) = exp(-0.5*ln(v+eps)): keeps ScalarE in the
            # natural_log_exp table set for the whole LN+softmax era (a Sqrt
            # here would cost ~2.7us of ACT table reload per switch).
            stats = stats_p.tile([P, 6], F32, tag="stats", name="stats")
            mv = stats_p.tile([P, 2], F32, tag="mv", name="mv")
            nc.vector.bn_stats(out=stats[:], in_=x_ap)
            nc.vector.bn_aggr(out=mv[:], in_=stats[:])
            sd = stats_p.tile([P, 1], F32, tag="sd", name="sd")
            nc.scalar.activation(out=sd[:], in_=mv[:, 1:2], func=AF.Ln,
                                 bias=eps_t[:])
            nc.scalar.activation(out=sd[:], in_=sd[:], func=AF.Exp,
                                 scale=-0.5)
            zt = lnt.tile([P, IN_C], out_dtype, tag=tag, name=tag)
            nc.vector.tensor_scalar(
                out=zt[:], in0=x_ap, scalar1=mv[:, 0:1], scalar2=sd[:],
                op0=ALU.subtract, op1=ALU.mult)
            return zt

        def emit_ln1(t):
            zt = ln_norm(xin[:, t * IN_C:(t + 1) * IN_C], BF16, "z16a")
            for k in range(2):
                tp = mm_ps.tile([P, P], BF16, tag="wk", name="tp")
                nc.tensor.transpose(tp[:], zt[:, k * P:(k + 1) * P], id16[:])
                dst = lnT[:, k * N + t * P: k * N + (t + 1) * P]
                if k == 0:
                    nc.vector.tensor_copy(out=dst, in_=tp[:])
                else:
                    nc.scalar.copy(out=dst, in_=tp[:])

        def emit_qproj(h, nn):
            ps = mm_ps.tile([P, 512], F32, tag="wk", name="proj")
            nc.tensor.matmul(
                ps[:], wq3[:, :, h * P:(h + 1) * P],
                lnT3[:, :, nn * 512:(nn + 1) * 512],
                start=True, stop=True, perf_mode=DR)
            if h % 2 == 0:
                nc.scalar.activation(
                    out=qT[h][:, nn * 512:(nn + 1) * 512], in_=ps[:],
                    func=AF.Identity, scale=SCALE, bias=bq_t[:, h:h + 1])
            else:
                nc.vector.tensor_scalar(
                    out=qT[h][:, nn * 512:(nn + 1) * 512], in0=ps[:],
                    scalar1=SCALE, scalar2=bq_t[:, h:h + 1],
                    op0=ALU.mult, op1=ALU.add)

        def emit_kproj(blk, nn):
            ps = mm_ps.tile([P, 512], F32, tag="wk", name="proj")
            nc.tensor.matmul(
                ps[:], wk3[:, :, blk * P:(blk + 1) * P],
                lnT3[:, :, nn * 512:(nn + 1) * 512],
                start=True, stop=True, perf_mode=DR)
            if (blk + nn) % 2 == 0:
                nc.scalar.copy(out=kT[blk][:, nn * 512:(nn + 1) * 512],
                               in_=ps[:])
            else:
                nc.vector.tensor_copy(out=kT[blk][:, nn * 512:(nn + 1) * 512],
                                      in_=ps[:])

        def emit_ve(h, t2):
            # two token tiles' values in one PSUM bank, one batched eviction
            # (alternating DVE/ACT so neither engine owns the whole stream)
            ps = mm_ps.tile([P, 2 * IN_C], F32, tag="wk", name="vps")
            for i in range(2):
                nc.tensor.matmul(
                    ps[:, i * IN_C:(i + 1) * IN_C],
                    lnT3[:, :, (2 * t2 + i) * P:(2 * t2 + i + 1) * P],
                    wvm3[:, :, h * IN_C:(h + 1) * IN_C],
                    start=True, stop=True, perf_mode=DR)
            src = ps[:].rearrange("p (t c) -> p t c", t=2)
            dst = ve3[h % 3][:, 2 * t2:2 * t2 + 2, 0:IN_C]
            if t2 % 2 == 0:
                nc.vector.tensor_copy(out=dst, in_=src)
            else:
                nc.scalar.copy(out=dst, in_=src)

        # Schraudolph bit-trick exp for the DVE/GpSimd-offloaded score tiles:
        # exp(S-4) ~ bitcast_f32(int32(A*S + B')); ~2-3% rms (fine: these keys'
        # softmax weights are fp8 anyway and attention is ~4% of the output).
        SCH_A = float(2.0 ** 23 / np.log(2.0))
        SCH_B = float(127 * 2 ** 23 - 4.0 * SCH_A - 486411.0)
        sch_p = ctx.enter_context(tc.tile_pool(name="sch", bufs=2))

        def emit_s(h, kc, eng="act"):
            blk = h // 2
            st = st_ps.tile([P, NQ], F32, tag="st", name="st")
            for half in range(2):  # one PSUM bank (512 f32) per matmul
                nc.tensor.matmul(
                    st[:, half * 512:(half + 1) * 512],
                    kT[blk][:, kc * P:(kc + 1) * P],
                    qT[h][:, half * 512:(half + 1) * 512],
                    start=True, stop=True)
            dst = exps[h % 3][:, kc * NQ:(kc + 1) * NQ]
            if eng == "act":
                nc.scalar.activation(out=dst, in_=st[:], func=AF.Exp,
                                     bias=cexp_t[:])
            else:
                # GPSIMD cannot touch PSUM (and only runs tensor_tensor-class
                # ops): DVE does the Schraudolph affine pass (PSUM -> SBUF
                # i32); the bitcast downcast runs on GpSimd ('pipe') or DVE
                # ('dve').
                sc = sch_p.tile([P, NQ], I32, tag=f"sch_{eng}", name="sch")
                nc.vector.tensor_scalar(
                    out=sc[:], in0=st[:], scalar1=SCH_A, scalar2=SCH_B,
                    op0=ALU.mult, op1=ALU.add)
                if eng == "pipe":
                    nc.gpsimd.tensor_copy(out=dst, in_=sc[:].bitcast(F32))
                else:
                    nc.vector.tensor_copy(out=dst, in_=sc[:].bitcast(F32))

        def emit_pv(h, qt):
            op = o_ps.tile([P, VE_W], F32, tag="opv", name="opv")
            for kc2 in range(NT_B // 2):
                nc.tensor.matmul(
                    op[:, 0:VE_W],
                    ex3[h % 3][:, 2 * kc2:2 * kc2 + 2, qt * P:(qt + 1) * P],
                    ve3[h % 3][:, 2 * kc2:2 * kc2 + 2, :],
                    start=(kc2 == 0), stop=(kc2 == NT_B // 2 - 1),
                    perf_mode=DR)
            rc = stats_p.tile([P, 1], F32, tag="rc", name="rc")
            nc.vector.reciprocal(out=rc[:], in_=op[:, IN_C:IN_C + 1])
            nc.vector.scalar_tensor_tensor(
                out=x2acc[qt][:], in0=op[:, 0:IN_C], scalar=rc[:],
                in1=x2acc[qt][:], op0=ALU.mult, op1=ALU.add)

        # ---- lead-in: LN1/transposes/projections/ve(0,1)/x2init, fused in
        # 4-tile groups so the PE has dense work while DVE/ACT chew LN ----
        for g in range(4):
            for t in range(4 * g, 4 * g + 4):
                emit_ln1(t)
            for blk in range(4):
                emit_kproj(blk, g)
            if g < 2:
                for h in range(H):
                    emit_qproj(h, g)
            # head-0 scores start as soon as kT[0] chunk g and all of qT[0]
            # exist: kc 4g..4g+3 need kT[0][:, kc*128] from chunk kc//4 <= g
            if g >= 1:
                for kc in range(4 * (g - 1), 4 * g):
                    emit_s(0, kc, eng=ENG_FOR(0, kc))
            for t2 in range(2 * g, 2 * g + 2):
                emit_ve(0, t2)
                emit_ve(1, t2)
            if g < 2:
                for qi in range(4 * g, 4 * g + 4):
                    nc.gpsimd.tensor_add(
                        out=x2acc[qi][:],
                        in0=xin[:, qi * IN_C:(qi + 1) * IN_C],
                        in1=battn_r[:])

        # ---- attention: software-pipelined over heads ----
        # Scores lead PV by two j-slots: head h+1's first four kc tiles are
        # emitted at the end of phase h (exps rotate through 3 buffers so
        # this is race-free), giving every eviction ~3 j-slots of slack
        # before PV(h, 0) consumes the full head.
        for kc in range(12, NT_B):
            emit_s(0, kc, eng=ENG_FOR(0, kc))
        for kc in range(4):
            emit_s(1, kc, eng=ENG_FOR(1, kc))
        for h in range(1, H):
            for j in range(NT_B // 2):
                if j < 6:
                    emit_s(h, 2 * j + 4, eng=ENG_FOR(h, 2 * j + 4))
                    emit_s(h, 2 * j + 5, eng=ENG_FOR(h, 2 * j + 5))
                    emit_pv(h - 1, j)
                else:
                    emit_pv(h - 1, j)
                    if h < H - 1:
                        kc0 = 2 * (j - 6)
                        emit_s(h + 1, kc0, eng=ENG_FOR(h + 1, kc0))
                        emit_s(h + 1, kc0 + 1, eng=ENG_FOR(h + 1, kc0 + 1))
                if h < H - 1:
                    emit_ve(h + 1, j)
        # ---- tail: PV(7) interleaved with LN2, then FFN ----
        def emit_ln2(qi):
            zt = ln_norm(x2acc[qi][:], BF16, "z16")
            for k in range(2):
                tp = mm_ps.tile([P, P], BF16, tag="wk", name="tp2")
                nc.tensor.transpose(tp[:], zt[:, k * P:(k + 1) * P], id16[:])
                nc.vector.tensor_copy(
                    out=x2T[:, k * NQ + qi * P: k * NQ + (qi + 1) * P],
                    in_=tp[:])
            nc.vector.tensor_add(out=x2acc[qi][:], in0=x2acc[qi][:],
                                 in1=b2_r[:])

        def emit_ffn1(eb, nh):
            ps = mm_ps.tile([P, 512], F32, tag="wk", name="proj")
            for k in range(2):
                nc.tensor.matmul(
                    ps[:], w1_t[:, k * EXPAND + eb * P: k * EXPAND + (eb + 1) * P],
                    x2T[:, k * NQ + nh * 512: k * NQ + (nh + 1) * 512],
                    start=(k == 0), stop=(k == 1))
            nc.scalar.activation(
                out=hT[:, eb * NQ + nh * 512: eb * NQ + (nh + 1) * 512],
                in_=ps[:], func=AF.Gelu, bias=b1_t[:, eb:eb + 1])

        def emit_ffn2(qi):
            op = o_ps.tile([P, IN_C], F32, tag="opv", name="fout")
            for eb in range(EXPAND // P):
                nc.tensor.matmul(
                    op[:, 0:IN_C],
                    hT[:, eb * NQ + qi * P: eb * NQ + (qi + 1) * P],
                    w2_t[:, eb * IN_C:(eb + 1) * IN_C],
                    start=(eb == 0), stop=(eb == EXPAND // P - 1))
            ot = on_p.tile([P, IN_C], F32, tag="out", name="out")
            nc.vector.tensor_add(out=ot[:], in0=op[:, 0:IN_C], in1=x2acc[qi][:])
            nc.sync.dma_start(out=out_es[qi][:], in_=ot[:])

        # tail: finish PV(7)/LN2 for the first query half, then stream the
        # second half's PV/LN2 between FFN1 chunks so ACT (gelu) and PE stay
        # co-busy; FFN2 of half 0 overlaps FFN1 of half 1.
        for qt in range(4):
            emit_pv(H - 1, qt)
            emit_ln2(qt)
        for eb in range(EXPAND // P):
            emit_ffn1(eb, 0)
            if eb < 4:
                emit_pv(H - 1, 4 + eb)
                emit_ln2(4 + eb)
        for qi in range(4):
            emit_ffn2(qi)
            emit_ffn1(2 * qi, 1)
            emit_ffn1(2 * qi + 1, 1)
        for qi in range(4, NT_Q):
            emit_ffn2(qi)

    if do_compile:
        nc.compile()
    return nc


def _to_bf16(a):
    return np.asarray(a, dtype=np.float32).astype(ml_dtypes.bfloat16)


def _to_fp8(a):
    return np.clip(np.asarray(a, dtype=np.float32), -240.0, 240.0).astype(
        ml_dtypes.float8_e4m3)


def _pairs(w):
    """[256, M] -> [128, 2*M] fp8 with chunk k at free offset k*M."""
    m = w.shape[1]
    out = np.empty((P, 2 * m), np.float32)
    out[:, :m] = w[:P, :]
    out[:, m:] = w[P:, :]
    return _to_fp8(out)


def kernel(x, ln1_g, ln1_b, Wqkv, bqkv, Wm, bm, ln2_g, ln2_b, W1, b1, W2, b2):
    global _NC
    x = np.asarray(x, dtype=np.float32)
    ln1_g = np.asarray(ln1_g, dtype=np.float32)
    ln1_b = np.asarray(ln1_b, dtype=np.float32)
    Wqkv = np.asarray(Wqkv, dtype=np.float32)
    bqkv = np.asarray(bqkv, dtype=np.float32)
    Wm = np.asarray(Wm, dtype=np.float32)
    bm = np.asarray(bm, dtype=np.float32)
    ln2_g = np.asarray(ln2_g, dtype=np.float32)
    ln2_b = np.asarray(ln2_b, dtype=np.float32)
    W1 = np.asarray(W1, dtype=np.float32)
    b1 = np.asarray(b1, dtype=np.float32)
    W2 = np.asarray(W2, dtype=np.float32)
    b2 = np.asarray(b2, dtype=np.float32)

    # fold LN1 gamma into all first-block weights; beta into biases
    Wg = ln1_g[:, None] * Wqkv
    # k: packed blocks, block b = heads 2b (rows 0:32) / 2b+1 (rows 64:96).
    # q: per-head zero-padded [128]-row layout matching the kT band, so the
    # score matmul can contract over all 128 rows in (128,128) tile mode.
    wq = np.zeros((IN_C, H * P), np.float32)
    wk = np.zeros((IN_C, 4 * P), np.float32)
    bq = np.zeros((P, H), np.float32)
    for h in range(H):
        blk, off = h // 2, 64 * (h % 2)
        wq[:, h * P + off: h * P + off + HEAD_C] = \
            Wg[:, h * HEAD_C:(h + 1) * HEAD_C]
        wk[:, blk * P + off: blk * P + off + HEAD_C] = \
            Wg[:, IN_C + h * HEAD_C: IN_C + (h + 1) * HEAD_C]
        # SCALE applied at eviction: psum*SCALE + bq
        bq[off:off + HEAD_C, h] = SCALE * (
            bqkv[h * HEAD_C:(h + 1) * HEAD_C]
            + ln1_b @ Wg[:, h * HEAD_C:(h + 1) * HEAD_C])
    # K bias dropped entirely: adds a per-query constant to S, softmax-invariant.
    wvm = np.empty((IN_C, H * IN_C), np.float32)
    battn = bm.copy()
    for h in range(H):
        Wv_h = Wg[:, 2 * IN_C + h * IN_C: 2 * IN_C + (h + 1) * IN_C]
        Wm_h = Wm[h * IN_C:(h + 1) * IN_C, :]
        wvm[:, h * IN_C:(h + 1) * IN_C] = 16.0 * (Wv_h @ Wm_h)
        battn += (bqkv[2 * IN_C + h * IN_C: 2 * IN_C + (h + 1) * IN_C]
                  + ln1_b @ Wv_h) @ Wm_h
    # fold LN2 gamma/beta into FFN1
    W1g = ln2_g[:, None] * W1
    b1_eff = b1 + ln2_b @ W1
    w1l = np.empty((P, 2 * EXPAND), np.float32)
    w1l[:, :EXPAND] = W1g[:P, :]
    w1l[:, EXPAND:] = W1g[P:, :]
    b1_l = np.ascontiguousarray(b1_eff.reshape(EXPAND // P, P).T).astype(np.float32)

    w2l = np.ascontiguousarray(
        W2.reshape(EXPAND // P, P, IN_C).transpose(1, 0, 2).reshape(P, -1))
    common = dict(
        wq=_pairs(wq), wk=_pairs(wk), wvm=_pairs(wvm),
        w1=_to_bf16(w1l), w2=_to_bf16(w2l),
        battn=battn.astype(np.float32), b2=b2, bq=bq, b1=b1_l,
    )
    in_maps = []
    for c in range(8):
        b, qh = c // 2, c % 2
        xb = np.concatenate(
            [x[b, qh * NQ:(qh + 1) * NQ, :], x[b, (1 - qh) * NQ:(2 - qh) * NQ, :]],
            axis=0)
        in_maps.append(dict(xb=np.ascontiguousarray(xb), **common))

    global LAST_RESULT
    if _NC is None:
        _NC = _build()
    res = run_bass_kernel_spmd(_NC, in_maps, core_ids=list(range(8)))
    LAST_RESULT = res  # exposes exec_time_ns when BASS_TRACE=1 is set
    out = np.concatenate(
        [np.asarray(res.results[c][f"out{i}"])
         for c in range(8) for i in range(NT_Q)], axis=0)
    return out.reshape(B, N, IN_C)


LAST_RESULT = None



# revision 69
# speedup vs baseline: 1.0769x; 1.0769x over previous
"""Trainium2 Bass kernel for nn_AttentionBlock (B=4, N=2048, IN_C=256, H=8,
HEAD_C=32, EXPAND=1024), distributed over 8 NeuronCores.

Sharding: data-parallel over query rows. Core c handles batch c//2, query rows
(c%2)*1024 .. +1024 of that batch. Each core receives its batch's tokens in
PERMUTED order (its own query rows first) so the query half of the LayerNorm /
transpose work doubles as the key/value half — softmax over keys is
permutation-invariant, so kT/ve just use the permuted order consistently.

Precision/engine strategy:
 - LN gains/betas are folded into the projection weights/biases host-side, so
   on-chip LN is just (x - mean) * rsqrt(var), evicted straight to fp8e4.
   rsqrt = exp(-0.5*ln(v+eps)) keeps ScalarE in ONE activation-table set
   (natural_log_exp, forced via the get_activation_tables patch below) for
   the whole LN+softmax program — table reloads cost ~1.5us each.
 - Q/K/V projections run as fp8 DoubleRow matmuls (2 k-tiles per pass).
 - Scores are bf16 in (128,128) tile mode: qT is per-head ZERO-PADDED to 128
   rows (row band 64*(h%2)..+32) so the score matmul contracts over the full
   128 rows of the packed kT[h//2] block — dead rows hit zeros. This keeps
   every PE matmul in the same tiling mode (no drain between scores and the
   fp8-DR PV/ve matmuls, which cost ~300ns each in the (32,128) variant).
 - Softmax: exp(S - 4) from PSUM to fp8e4 (S max ~8.6 so exp stays below the
   e4m3 240 cap; the -4 offset cancels in normalization). The eviction is
   load-balanced per key tile: ScalarE exp for most tiles, a Schraudolph
   bitcast-exp (i32 affine on DVE, downcast on GpSimd or DVE) for the rest —
   GPSIMD cannot read PSUM, so DVE always runs the PSUM-side pass.
 - PV runs as fp8 DoubleRow over key-tile pairs; the appended "ones" column
   is set to 16.0, cancelling the x16 scaling folded into wvm (which keeps
   wvm out of the fp8 subnormal range) at normalization time.
 - FFN stays bf16 end to end for accuracy.
 - Heads are software-pipelined with scores leading PV by two j-slots (exps
   rotate through 3 buffers): every eviction has ~3 slots of queue slack
   before PV(h, 0) consumes the full head, hiding ScalarE/GpSimd latency.
 - Tail streams the second query half's PV/LN2 between FFN1 chunks so gelu
   (ScalarE) and the PE stay co-busy.
"""
import numpy as np
import ml_dtypes

import concourse.bass as bass
import concourse.bacc as bacc
import concourse.tile as tile
from concourse import mybir
from concourse.bass_utils import run_bass_kernel_spmd
from concourse.masks import make_identity

F32 = mybir.dt.float32
BF16 = mybir.dt.bfloat16
FP8 = mybir.dt.float8e4
I32 = mybir.dt.int32
AF = mybir.ActivationFunctionType
ALU = mybir.AluOpType
DR = mybir.MatmulPerfMode.DoubleRow
P = 128

# The act-table insertion pass picks the FIRST table set containing each
# activation function. Ln lives in `natural_log` (set 5) and Exp in
# `exp_and_others` (set 0), so a kernel alternating Ln/Exp reloads tables
# (~1.5us each) on every switch. Restrict membership so both resolve to the
# combined `natural_log_exp_and_others` set: the emitted act_func_set_id
# still indexes the real act_info.json list, so runtime tables are correct.
_GAT_PATCHED = False


def _patch_act_tables():
    global _GAT_PATCHED
    if _GAT_PATCHED:
        return
    orig = bacc.get_activation_tables

    def patched(arch):
        t = dict(orig(arch))
        for name in ("exp_and_others", "exp_and_friends"):
            if name in t:
                t[name] = set(t[name]) - {AF.Exp}
        if "natural_log" in t:
            t["natural_log"] = set(t["natural_log"]) - {AF.Ln}
        return t

    bacc.get_activation_tables = patched
    _GAT_PATCHED = True

B, N, IN_C, HEAD_C, H, EXPAND = 4, 2048, 256, 32, 8, 1024
NQ = 1024            # query rows per core

# softmax-exp engine assignment per key tile: Schraudolph offload relieves
# ScalarE, which is otherwise the attention-era bottleneck. 'pipe' = DVE
# affine pass + GpSimd downcast; 'dve' = both passes on DVE.
# offloaded kc spread so no long ACT runs; late kc (15) and the prefetched
# boundary tiles (0, 1, 3) stay on the fast ACT path.
_PIPE_KC = frozenset({2, 5, 8, 11})
_DVE_KC = frozenset({4, 14})


def ENG_FOR(h, kc):
    if kc in _PIPE_KC:
        return "pipe"
    if kc in _DVE_KC:
        return "dve"
    return "act"
NT_B = N // P        # 16 token tiles per batch
NT_Q = NQ // P       # 8 query tiles per core
LN_EPS = 1e-5
C_EXP = 4.0          # exp offset: exp(S - C_EXP) <= ~110 < 240 (e4m3 max)
VE_W = IN_C + 1      # 257: value channels + sum column (holds 16.0)
SCALE = HEAD_C ** -0.5

_NC = None  # cached compiled graph


def _build(do_compile=True):
    _patch_act_tables()
    nc = bacc.Bacc("TRN2", target_bir_lowering=False, debug=False, num_devices=8)

    xb_e = nc.dram_tensor("xb", [N, IN_C], F32, kind="ExternalInput")
    # fp8 weights, contraction-pair layout: free dim = (k, cols), k in {0,1}
    # selecting input-channel chunk k*128..+128.
    wq_e = nc.dram_tensor("wq", [P, 2 * H * P], FP8, kind="ExternalInput")
    wk_e = nc.dram_tensor("wk", [P, 2 * 4 * P], FP8, kind="ExternalInput")
    wvm_e = nc.dram_tensor("wvm", [P, 2 * H * IN_C], FP8, kind="ExternalInput")
    w1_e = nc.dram_tensor("w1", [P, 2 * EXPAND], BF16, kind="ExternalInput")
    w2_e = nc.dram_tensor("w2", [P, EXPAND // P * IN_C], BF16, kind="ExternalInput")
    battn_e = nc.dram_tensor("battn", [IN_C], F32, kind="ExternalInput")
    b2_e = nc.dram_tensor("b2", [IN_C], F32, kind="ExternalInput")
    bq_e = nc.dram_tensor("bq", [P, H], F32, kind="ExternalInput")
    b1_e = nc.dram_tensor("b1", [P, EXPAND // P], F32, kind="ExternalInput")
    out_es = [nc.dram_tensor(f"out{i}", [P, IN_C], F32, kind="ExternalOutput")
              for i in range(NT_Q)]

    from contextlib import ExitStack
    with tile.TileContext(nc) as tc, ExitStack() as ctx:
        const = ctx.enter_context(tc.tile_pool(name="const", bufs=1))
        persist = ctx.enter_context(tc.tile_pool(name="persist", bufs=1))
        lnt = ctx.enter_context(tc.tile_pool(name="lnt", bufs=3))
        stats_p = ctx.enter_context(tc.tile_pool(name="stats", bufs=4))
        on_p = ctx.enter_context(tc.tile_pool(name="on", bufs=3))
        st_ps = ctx.enter_context(tc.tile_pool(name="st_ps", bufs=2, space="PSUM"))
        o_ps = ctx.enter_context(tc.tile_pool(name="o_ps", bufs=2, space="PSUM"))
        mm_ps = ctx.enter_context(tc.tile_pool(name="mm_ps", bufs=2, space="PSUM"))

        # ---- input + early-needed weights first: the LN1 chain starts on
        # xin immediately and kproj/qproj need wk/wq within ~3us; wvm/w1/w2
        # are needed later and must not crowd the queues. ----
        wq_t = const.tile([P, 2 * H * P], FP8, tag="wq", name="wq")
        nc.sync.dma_start(out=wq_t[:], in_=wq_e[:])
        wk_t = const.tile([P, 2 * 4 * P], FP8, tag="wk", name="wk")
        nc.sync.dma_start(out=wk_t[:], in_=wk_e[:])
        xin = persist.tile([P, NT_B * IN_C], F32, tag="xin", name="xin")
        for t in range(NT_B):
            nc.sync.dma_start(
                out=xin[:, t * IN_C:(t + 1) * IN_C],
                in_=xb_e[t * P:(t + 1) * P, :])

        # ---- constants ----
        def rep256(name, ext):
            t = const.tile([P, IN_C], F32, tag=name, name=name)
            nc.sync.dma_start(
                out=t[:], in_=ext.ap()[None, :].broadcast_to([P, IN_C]))
            return t

        battn_r = rep256("battn", battn_e)
        b2_r = rep256("b2", b2_e)
        bq_t = const.tile([P, H], F32, tag="bq", name="bq")
        nc.sync.dma_start(out=bq_t[:], in_=bq_e[:])
        b1_t = const.tile([P, EXPAND // P], F32, tag="b1", name="b1")
        nc.sync.dma_start(out=b1_t[:], in_=b1_e[:])
        eps_t = const.tile([P, 1], F32, tag="eps", name="eps")
        nc.vector.memset(eps_t[:], LN_EPS)
        cexp_t = const.tile([P, 1], F32, tag="cexp", name="cexp")
        nc.vector.memset(cexp_t[:], -C_EXP)
        id16 = const.tile([P, P], BF16, tag="id16", name="id16")
        make_identity(nc, id16[:])

        wvm_t = const.tile([P, 2 * H * IN_C], FP8, tag="wvm", name="wvm")
        nc.sync.dma_start(out=wvm_t[:], in_=wvm_e[:])
        w1_t = const.tile([P, 2 * EXPAND], BF16, tag="w1", name="w1")
        nc.sync.dma_start(out=w1_t[:], in_=w1_e[:])
        w2_t = const.tile([P, EXPAND // P * IN_C], BF16, tag="w2", name="w2")
        nc.sync.dma_start(out=w2_t[:], in_=w2_e[:])
        wq3 = wq_t[:].rearrange("p (k n) -> p k n", k=2)
        wk3 = wk_t[:].rearrange("p (k n) -> p k n", k=2)
        wvm3 = wvm_t[:].rearrange("p (k n) -> p k n", k=2)

        # ---- persistent activations ----
        lnT = persist.tile([P, 2 * N], FP8, tag="lnT", name="lnT")
        lnT3 = lnT[:].rearrange("p (k n) -> p k n", k=2)
        # per-head zero-padded qT: head h occupies rows 64*(h%2)..+32, rest 0.
        # Scores then contract over the FULL 128 rows of packed kT[h//2]
        # (whose other rows hold the partner head / zeros), keeping every
        # matmul in (128,128) tile mode - no PE tiling-mode drains.
        qT = [persist.tile([P, NQ], BF16, tag=f"qT{h}", name=f"qT{h}")
              for h in range(H)]
        kT = [persist.tile([P, N], BF16, tag=f"kT{k}", name=f"kT{k}")
              for k in range(4)]
        x2acc = [persist.tile([P, IN_C], F32, tag=f"x2acc{i}", name=f"x2acc{i}")
                 for i in range(NT_Q)]
        x2T = persist.tile([P, 2 * NQ], BF16, tag="x2T", name="x2T")
        hT = persist.tile([P, EXPAND // P * NQ], BF16, tag="hT", name="hT")
        exps = [persist.tile([P, NT_B * NQ], FP8, tag=f"exps{i}", name=f"exps{i}")
                for i in range(3)]
        ve = [persist.tile([P, NT_B * VE_W], FP8, tag=f"ve{i}", name=f"ve{i}")
              for i in range(3)]
        # sum columns hold 16.0: cancels the x16 folded into wvm at normalize
        for i in range(3):
            nc.gpsimd.memset(
                ve[i][:].rearrange("p (t c) -> p t c", c=VE_W)[:, :, IN_C:IN_C + 1],
                16.0)
        ex3 = [e[:].rearrange("p (k n) -> p k n", k=NT_B) for e in exps]
        ve3 = [v[:].rearrange("p (t c) -> p t c", c=VE_W) for v in ve]



        # ---- LayerNorm: stats on DVE, sqrt on ACT, normalize straight to out
        def ln_norm(x_ap, out_dtype, tag):
            # rsqrt(v+eps) = exp(-0.5*ln(v+eps)): keeps ScalarE in the
            # natural_log_exp table set for the whole LN+softmax era (a Sqrt
            # here would cost ~2.7us of ACT table reload per switch).
            stats = stats_p.tile([P, 6], F32, tag="stats", name="stats")
            mv = stats_p.tile([P, 2], F32, tag="mv", name="mv")
            nc.vector.bn_stats(out=stats[:], in_=x_ap)
            nc.vector.bn_aggr(out=mv[:], in_=stats[:])
            sd = stats_p.tile([P, 1], F32, tag="sd", name="sd")
            nc.scalar.activation(out=sd[:], in_=mv[:, 1:2], func=AF.Ln,
                                 bias=eps_t[:])
            nc.scalar.activation(out=sd[:], in_=sd[:], func=AF.Exp,
                                 scale=-0.5)
            zt = lnt.tile([P, IN_C], out_dtype, tag=tag, name=tag)
            nc.vector.tensor_scalar(
                out=zt[:], in0=x_ap, scalar1=mv[:, 0:1], scalar2=sd[:],
                op0=ALU.subtract, op1=ALU.mult)
            return zt

        def emit_ln1(t):
            zt = ln_norm(xin[:, t * IN_C:(t + 1) * IN_C], BF16, "z16a")
            for k in range(2):
                tp = mm_ps.tile([P, P], BF16, tag="wk", name="tp")
                nc.tensor.transpose(tp[:], zt[:, k * P:(k + 1) * P], id16[:])
                dst = lnT[:, k * N + t * P: k * N + (t + 1) * P]
                if k == 0:
                    nc.vector.tensor_copy(out=dst, in_=tp[:])
                else:
                    nc.scalar.copy(out=dst, in_=tp[:])

        def emit_qproj(h, nn):
            ps = mm_ps.tile([P, 512], F32, tag="wk", name="proj")
            nc.tensor.matmul(
                ps[:], wq3[:, :, h * P:(h + 1) * P],
                lnT3[:, :, nn * 512:(nn + 1) * 512],
                start=True, stop=True, perf_mode=DR)
            if h % 2 == 0:
                nc.scalar.activation(
                    out=qT[h][:, nn * 512:(nn + 1) * 512], in_=ps[:],
                    func=AF.Identity, scale=SCALE, bias=bq_t[:, h:h + 1])
            else:
                nc.vector.tensor_scalar(
                    out=qT[h][:, nn * 512:(nn + 1) * 512], in0=ps[:],
                    scalar1=SCALE, scalar2=bq_t[:, h:h + 1],
                    op0=ALU.mult, op1=ALU.add)

        def emit_kproj(blk, nn):
            ps = mm_ps.tile([P, 512], F32, tag="wk", name="proj")
            nc.tensor.matmul(
                ps[:], wk3[:, :, blk * P:(blk + 1) * P],
                lnT3[:, :, nn * 512:(nn + 1) * 512],
                start=True, stop=True, perf_mode=DR)
            if (blk + nn) % 2 == 0:
                nc.scalar.copy(out=kT[blk][:, nn * 512:(nn + 1) * 512],
                               in_=ps[:])
            else:
                nc.vector.tensor_copy(out=kT[blk][:, nn * 512:(nn + 1) * 512],
                                      in_=ps[:])

        def emit_ve(h, t2):
            # two token tiles' values in one PSUM bank, one batched eviction
            # (alternating DVE/ACT so neither engine owns the whole stream)
            ps = mm_ps.tile([P, 2 * IN_C], F32, tag="wk", name="vps")
            for i in range(2):
                nc.tensor.matmul(
                    ps[:, i * IN_C:(i + 1) * IN_C],
                    lnT3[:, :, (2 * t2 + i) * P:(2 * t2 + i + 1) * P],
                    wvm3[:, :, h * IN_C:(h + 1) * IN_C],
                    start=True, stop=True, perf_mode=DR)
            src = ps[:].rearrange("p (t c) -> p t c", t=2)
            dst = ve3[h % 3][:, 2 * t2:2 * t2 + 2, 0:IN_C]
            if t2 % 2 == 0:
                nc.vector.tensor_copy(out=dst, in_=src)
            else:
                nc.scalar.copy(out=dst, in_=src)

        # Schraudolph bit-trick exp for the DVE/GpSimd-offloaded score tiles:
        # exp(S-4) ~ bitcast_f32(int32(A*S + B')); ~2-3% rms (fine: these keys'
        # softmax weights are fp8 anyway and attention is ~4% of the output).
        SCH_A = float(2.0 ** 23 / np.log(2.0))
        SCH_B = float(127 * 2 ** 23 - 4.0 * SCH_A - 486411.0)
        sch_p = ctx.enter_context(tc.tile_pool(name="sch", bufs=2))

        def emit_s(h, kc, eng="act"):
            blk = h // 2
            st = st_ps.tile([P, NQ], F32, tag="st", name="st")
            for half in range(2):  # one PSUM bank (512 f32) per matmul
                nc.tensor.matmul(
                    st[:, half * 512:(half + 1) * 512],
                    kT[blk][:, kc * P:(kc + 1) * P],
                    qT[h][:, half * 512:(half + 1) * 512],
                    start=True, stop=True)
            dst = exps[h % 3][:, kc * NQ:(kc + 1) * NQ]
            if eng == "act":
                nc.scalar.activation(out=dst, in_=st[:], func=AF.Exp,
                                     bias=cexp_t[:])
            else:
                # GPSIMD cannot touch PSUM (and only runs tensor_tensor-class
                # ops): DVE does the Schraudolph affine pass (PSUM -> SBUF
                # i32); the bitcast downcast runs on GpSimd ('pipe') or DVE
                # ('dve').
                # 4 bufs for the GpSimd pipe: its 3.6us downcast otherwise
                # blocks the NEXT pass1 at the head of the DVE FIFO
                # (head-of-line blocking behind a 2-buf rotation).
                sc = sch_p.tile([P, NQ], I32, tag=f"sch_{eng}", name="sch",
                                bufs=4 if eng == "pipe" else 2)
                nc.vector.tensor_scalar(
                    out=sc[:], in0=st[:], scalar1=SCH_A, scalar2=SCH_B,
                    op0=ALU.mult, op1=ALU.add)
                if eng == "pipe":
                    nc.gpsimd.tensor_copy(out=dst, in_=sc[:].bitcast(F32))
                else:
                    nc.vector.tensor_copy(out=dst, in_=sc[:].bitcast(F32))

        def emit_pv(h, qt):
            op = o_ps.tile([P, VE_W], F32, tag="opv", name="opv")
            for kc2 in range(NT_B // 2):
                nc.tensor.matmul(
                    op[:, 0:VE_W],
                    ex3[h % 3][:, 2 * kc2:2 * kc2 + 2, qt * P:(qt + 1) * P],
                    ve3[h % 3][:, 2 * kc2:2 * kc2 + 2, :],
                    start=(kc2 == 0), stop=(kc2 == NT_B // 2 - 1),
                    perf_mode=DR)
            rc = stats_p.tile([P, 1], F32, tag="rc", name="rc")
            nc.vector.reciprocal(out=rc[:], in_=op[:, IN_C:IN_C + 1])
            nc.vector.scalar_tensor_tensor(
                out=x2acc[qt][:], in0=op[:, 0:IN_C], scalar=rc[:],
                in1=x2acc[qt][:], op0=ALU.mult, op1=ALU.add)

        # ---- lead-in: LN1/transposes/projections/ve(0,1)/x2init, fused in
        # 4-tile groups so the PE has dense work while DVE/ACT chew LN ----
        for g in range(4):
            for t in range(4 * g, 4 * g + 4):
                emit_ln1(t)
            for blk in range(4):
                emit_kproj(blk, g)
            if g < 2:
                for h in range(H):
                    emit_qproj(h, g)
            # head-0 scores start as soon as kT[0] chunk g and all of qT[0]
            # exist: kc 4g..4g+3 need kT[0][:, kc*128] from chunk kc//4 <= g
            if g >= 1:
                for kc in range(4 * (g - 1), 4 * g):
                    emit_s(0, kc, eng=ENG_FOR(0, kc))
            for t2 in range(2 * g, 2 * g + 2):
                emit_ve(0, t2)
                emit_ve(1, t2)
            if g < 2:
                for qi in range(4 * g, 4 * g + 4):
                    nc.gpsimd.tensor_add(
                        out=x2acc[qi][:],
                        in0=xin[:, qi * IN_C:(qi + 1) * IN_C],
                        in1=battn_r[:])

        # ---- attention: software-pipelined over heads ----
        # Scores lead PV by two j-slots: head h+1's first four kc tiles are
        # emitted at the end of phase h (exps rotate through 3 buffers so
        # this is race-free), giving every eviction ~3 j-slots of slack
        # before PV(h, 0) consumes the full head.
        for kc in range(12, NT_B):
            emit_s(0, kc, eng=ENG_FOR(0, kc))
        for kc in range(4):
            emit_s(1, kc, eng=ENG_FOR(1, kc))
        for h in range(1, H):
            for j in range(NT_B // 2):
                if j < 6:
                    emit_s(h, 2 * j + 4, eng=ENG_FOR(h, 2 * j + 4))
                    emit_s(h, 2 * j + 5, eng=ENG_FOR(h, 2 * j + 5))
                    emit_pv(h - 1, j)
                else:
                    emit_pv(h - 1, j)
                    if h < H - 1:
                        kc0 = 2 * (j - 6)
                        emit_s(h + 1, kc0, eng=ENG_FOR(h + 1, kc0))
                        emit_s(h + 1, kc0 + 1, eng=ENG_FOR(h + 1, kc0 + 1))
                if h < H - 1:
                    emit_ve(h + 1, j)
        # ---- tail: PV(7) interleaved with LN2, then FFN ----
        def emit_ln2(qi):
            zt = ln_norm(x2acc[qi][:], BF16, "z16")
            for k in range(2):
                tp = mm_ps.tile([P, P], BF16, tag="wk", name="tp2")
                nc.tensor.transpose(tp[:], zt[:, k * P:(k + 1) * P], id16[:])
                nc.vector.tensor_copy(
                    out=x2T[:, k * NQ + qi * P: k * NQ + (qi + 1) * P],
                    in_=tp[:])
            nc.vector.tensor_add(out=x2acc[qi][:], in0=x2acc[qi][:],
                                 in1=b2_r[:])

        def emit_ffn1(eb, nh):
            ps = mm_ps.tile([P, 512], F32, tag="wk", name="proj")
            for k in range(2):
                nc.tensor.matmul(
                    ps[:], w1_t[:, k * EXPAND + eb * P: k * EXPAND + (eb + 1) * P],
                    x2T[:, k * NQ + nh * 512: k * NQ + (nh + 1) * 512],
                    start=(k == 0), stop=(k == 1))
            nc.scalar.activation(
                out=hT[:, eb * NQ + nh * 512: eb * NQ + (nh + 1) * 512],
                in_=ps[:], func=AF.Gelu, bias=b1_t[:, eb:eb + 1])

        def emit_ffn2(qi):
            op = o_ps.tile([P, IN_C], F32, tag="opv", name="fout")
            for eb in range(EXPAND // P):
                nc.tensor.matmul(
                    op[:, 0:IN_C],
                    hT[:, eb * NQ + qi * P: eb * NQ + (qi + 1) * P],
                    w2_t[:, eb * IN_C:(eb + 1) * IN_C],
                    start=(eb == 0), stop=(eb == EXPAND // P - 1))
            ot = on_p.tile([P, IN_C], F32, tag="out", name="out")
            nc.vector.tensor_add(out=ot[:], in0=op[:, 0:IN_C], in1=x2acc[qi][:])
            nc.sync.dma_start(out=out_es[qi][:], in_=ot[:])

        # tail: finish PV(7)/LN2 for the first query half, then stream the
        # second half's PV/LN2 between FFN1 chunks so ACT (gelu) and PE stay
        # co-busy; FFN2 of half 0 overlaps FFN1 of half 1.
        for qt in range(4):
            emit_pv(H - 1, qt)
            emit_ln2(qt)
        for eb in range(EXPAND // P):
            emit_ffn1(eb, 0)
            if eb < 4:
                emit_pv(H - 1, 4 + eb)
                emit_ln2(4 + eb)
        for qi in range(4):
            emit_ffn2(qi)
            emit_ffn1(2 * qi, 1)
            emit_ffn1(2 * qi + 1, 1)
        for qi in range(4, NT_Q):
            emit_ffn2(qi)

    if do_compile:
        nc.compile()
    return nc


def _to_bf16(a):
    return np.asarray(a, dtype=np.float32).astype(ml_dtypes.bfloat16)


def _to_fp8(a):
    return np.clip(np.asarray(a, dtype=np.float32), -240.0, 240.0).astype(
        ml_dtypes.float8_e4m3)


def _pairs(w):
    """[256, M] -> [128, 2*M] fp8 with chunk k at free offset k*M."""
    m = w.shape[1]
    out = np.empty((P, 2 * m), np.float32)
    out[:, :m] = w[:P, :]
    out[:, m:] = w[P:, :]
    return _to_fp8(out)


def kernel(x, ln1_g, ln1_b, Wqkv, bqkv, Wm, bm, ln2_g, ln2_b, W1, b1, W2, b2):
    global _NC
    x = np.asarray(x, dtype=np.float32)
    ln1_g = np.asarray(ln1_g, dtype=np.float32)
    ln1_b = np.asarray(ln1_b, dtype=np.float32)
    Wqkv = np.asarray(Wqkv, dtype=np.float32)
    bqkv = np.asarray(bqkv, dtype=np.float32)
    Wm = np.asarray(Wm, dtype=np.float32)
    bm = np.asarray(bm, dtype=np.float32)
    ln2_g = np.asarray(ln2_g, dtype=np.float32)
    ln2_b = np.asarray(ln2_b, dtype=np.float32)
    W1 = np.asarray(W1, dtype=np.float32)
    b1 = np.asarray(b1, dtype=np.float32)
    W2 = np.asarray(W2, dtype=np.float32)
    b2 = np.asarray(b2, dtype=np.float32)

    # fold LN1 gamma into all first-block weights; beta into biases
    Wg = ln1_g[:, None] * Wqkv
    # k: packed blocks, block b = heads 2b (rows 0:32) / 2b+1 (rows 64:96).
    # q: per-head zero-padded [128]-row layout matching the kT band, so the
    # score matmul can contract over all 128 rows in (128,128) tile mode.
    wq = np.zeros((IN_C, H * P), np.float32)
    wk = np.zeros((IN_C, 4 * P), np.float32)
    bq = np.zeros((P, H), np.float32)
    for h in range(H):
        blk, off = h // 2, 64 * (h % 2)
        wq[:, h * P + off: h * P + off + HEAD_C] = \
            Wg[:, h * HEAD_C:(h + 1) * HEAD_C]
        wk[:, blk * P + off: blk * P + off + HEAD_C] = \
            Wg[:, IN_C + h * HEAD_C: IN_C + (h + 1) * HEAD_C]
        # SCALE applied at eviction: psum*SCALE + bq
        bq[off:off + HEAD_C, h] = SCALE * (
            bqkv[h * HEAD_C:(h + 1) * HEAD_C]
            + ln1_b @ Wg[:, h * HEAD_C:(h + 1) * HEAD_C])
    # K bias dropped entirely: adds a per-query constant to S, softmax-invariant.
    wvm = np.empty((IN_C, H * IN_C), np.float32)
    battn = bm.copy()
    for h in range(H):
        Wv_h = Wg[:, 2 * IN_C + h * IN_C: 2 * IN_C + (h + 1) * IN_C]
        Wm_h = Wm[h * IN_C:(h + 1) * IN_C, :]
        wvm[:, h * IN_C:(h + 1) * IN_C] = 16.0 * (Wv_h @ Wm_h)
        battn += (bqkv[2 * IN_C + h * IN_C: 2 * IN_C + (h + 1) * IN_C]
                  + ln1_b @ Wv_h) @ Wm_h
    # fold LN2 gamma/beta into FFN1
    W1g = ln2_g[:, None] * W1
    b1_eff = b1 + ln2_b @ W1
    w1l = np.empty((P, 2 * EXPAND), np.float32)
    w1l[:, :EXPAND] = W1g[:P, :]
    w1l[:, EXPAND:] = W1g[P:, :]
    b1_l = np.ascontiguousarray(b1_eff.reshape(EXPAND // P, P).T).astype(np.float32)

    w2l = np.ascontiguousarray(
        W2.reshape(EXPAND // P, P, IN_C).transpose(1, 0, 2).reshape(P, -1))
    common = dict(
        wq=_pairs(wq), wk=_pairs(wk), wvm=_pairs(wvm),
        w1=_to_bf16(w1l), w2=_to_bf16(w2l),
        battn=battn.astype(np.float32), b2=b2, bq=bq, b1=b1_l,
    )
    in_maps = []
    for c in range(8):
        b, qh = c // 2, c % 2
        xb = np.concatenate(
            [x[b, qh * NQ:(qh + 1) * NQ, :], x[b, (1 - qh) * NQ:(2 - qh) * NQ, :]],
            axis=0)
        in_maps.append(dict(xb=np.ascontiguousarray(xb), **common))

    global LAST_RESULT
    if _NC is None:
        _NC = _build()
    res = run_bass_kernel_spmd(_NC, in_maps, core_ids=list(range(8)))
    LAST_RESULT = res  # exposes exec_time_ns when BASS_TRACE=1 is set
    out = np.concatenate(
        [np.asarray(res.results[c][f"out{i}"])
         for c in range(8) for i in range(NT_Q)], axis=0)
    return out.reshape(B, N, IN_C)


LAST_RESULT = None

